# revision 16
# baseline (speedup 1.0000x reference)
"""Trainium2 Bass kernel for nn_Decoder (single-step attention decoder).

Sharding over 8 NeuronCores:
  - LSTM: gate-interleaved row shard (core k computes h/c slice [128k:128k+128])
  - score linear: A-term (ht part) input-sharded, B-term (encoder part)
    sequence-sharded; partial softmax with global renorm via AllGather
  - W_attn: replicated (bf16), ht_tilda computed fully on every core
  - W_out: vocab-sharded (4000 rows/core, padded to 4096); log-softmax via
    local sum-exp + tiny AllGather of partial Z
All weight matmuls run in bf16; accumulation/bias/softmax math in fp32.
Host-side work is limited to slicing / transposing / casting inputs and
reassembling outputs (the embedding row gather is part of sharding).
"""

import numpy as np
import ml_dtypes

V = 32000
D = 1024
H = 1024
E = 1024
S = 4096
NCORES = 8
HS = H // NCORES          # 128  hidden slice
SS = S // NCORES          # 512  sequence slice
VS = V // NCORES          # 4000 vocab slice
VSP = 4096                # padded vocab slice (32 x 128)

BF16 = ml_dtypes.bfloat16

_CACHE = {}


def _build_program(collectives=True, stage=5):
    nc = _build_body(collectives, stage)
    nc.compile()
    return nc


def _build_body(collectives=True, stage=5):
    import concourse.bacc as bacc
    import concourse.bass as bass
    import concourse.mybir as mybir
    import concourse.tile as tile

    f32 = mybir.dt.float32
    bf16 = mybir.dt.bfloat16
    AF = mybir.ActivationFunctionType

    nc = bacc.Bacc(
        "TRN2",
        target_bir_lowering=False,
        debug=False,
        num_devices=NCORES,
    )

    # ---------------- DRAM I/O ----------------
    din = {}

    def inp(name, shape, dt=bf16):
        din[name] = nc.dram_tensor(name, list(shape), dt, kind="ExternalInput").ap()
        return din[name]

    z_col = inp("z_col", [128, 16])                # [x; h0] column chunks
    wk_T = inp("wk_T", [2048, 512])                # LSTM weight slice (in-major)
    bk_col = inp("bk_col", [128, 4], f32)          # per-gate bias cols
    c0_col = inp("c0_col", [128, 1], f32)
    wl_T = inp("wl_T", [128, 1024])                # W_score[:, :H][:, hslice].T
    bs8_col = inp("bs8_col", [128, 8], f32)        # b_score/8 cols
    wr_T = inp("wr_T", [1024, 1024])               # W_score[:, H:].T (replicated)
    encT = inp("encT", [1024, 512])                # encoder slice transposed
    wva_col = inp("wva_col", [128, 8])             # w_va column chunks
    wattn_T = inp("wattn_T", [2048, 1024])         # W_attn.T (replicated)
    battn_col = inp("battn_col", [128, 8], f32)
    wout_T = inp("wout_T", [1024, VSP])            # W_out[vslice].T, padded
    bout_col = inp("bout_col", [128, 32], f32)     # b_out cols (pad = -1e30)
    ones_row = inp("ones_row", [1, 128], f32)
    negones_row = inp("negones_row", [1, 128], f32)
    ones_col = inp("ones_col", [128, 1], f32)
    ones128 = inp("ones128", [128, 128], f32)
    ident = inp("ident", [128, 128], f32)

    out_logits = nc.dram_tensor("out_logits", [32, 128], f32, kind="ExternalOutput").ap()
    out_h = nc.dram_tensor("out_h", [128, 1], f32, kind="ExternalOutput").ap()
    out_c = nc.dram_tensor("out_c", [128, 1], f32, kind="ExternalOutput").ap()
    out_attn = nc.dram_tensor("out_attn", [1, 512], f32, kind="ExternalOutput").ap()

    RG = [list(range(NCORES))]

    with tile.TileContext(nc) as tc:
        with (
            tc.tile_pool(name="wpool", bufs=1) as wpool,
            tc.tile_pool(name="spool", bufs=1) as spool,
            tc.tile_pool(name="dram", bufs=1, space="DRAM") as dram,
            tc.tile_pool(name="psA", bufs=1, space="PSUM") as psA,
            tc.tile_pool(name="psB", bufs=2, space="PSUM") as psB,
            tc.tile_pool(name="psC", bufs=1, space="PSUM") as psC,
            tc.tile_pool(name="psD", bufs=1, space="PSUM") as psD,
        ):
            # ---------- weight / constant loads (emitted in consumption order) ----------
            def load(pool, ap, shape, dt=bf16, name=None):
                t = pool.tile(shape, dt, name=name, uniquify=True)
                nc.sync.dma_start(t[:], ap)
                return t

            z_sb = load(spool, z_col[:, :], [128, 16], name="z_sb")
            bk_sb = load(spool, bk_col[:, :], [128, 4], f32, name="bk_sb")
            c0_sb = load(spool, c0_col[:, :], [128, 1], f32, name="c0_sb")
            wk_sb = [
                load(wpool, wk_T[128 * q : 128 * (q + 1), :], [128, 512], name=f"wk{q}")
                for q in range(16)
            ]
            wl_sb = load(wpool, wl_T[:, :], [128, 1024], name="wl_sb")
            bs8_sb = load(spool, bs8_col[:, :], [128, 8], f32, name="bs8_sb")
            ones_sb = load(spool, ones_row[:, :], [1, 128], f32, name="ones_sb")
            negones_sb = load(spool, negones_row[:, :], [1, 128], f32, name="negones_sb")
            onesc_sb = load(spool, ones_col[:, :], [128, 1], f32, name="onesc_sb")
            ones128_sb = load(spool, ones128[:, :], [128, 128], f32, name="ones128_sb")
            ident_sb = load(spool, ident[:, :], [128, 128], f32, name="ident_sb")
            wva_sb = load(spool, wva_col[:, :], [128, 8], name="wva_sb")
            wr_sb = [
                load(wpool, wr_T[128 * q : 128 * (q + 1), :], [128, 1024], name=f"wr{q}")
                for q in range(8)
            ]
            enc_sb = [
                load(wpool, encT[128 * q : 128 * (q + 1), :], [128, 512], name=f"enc{q}")
                for q in range(8)
            ]
            wattn_sb = [
                load(wpool, wattn_T[128 * q : 128 * (q + 1), :], [128, 1024], name=f"wat{q}")
                for q in range(16)
            ]
            battn_sb = load(spool, battn_col[:, :], [128, 8], f32, name="battn_sb")
            bout_sb = load(spool, bout_col[:, :], [128, 32], f32, name="bout_sb")
            wout_sb = [
                load(wpool, wout_T[128 * q : 128 * (q + 1), :], [128, VSP], name=f"wo{q}")
                for q in range(8)
            ]

            # ---------- LSTM step (local 128-slice of h and c) ----------
            ps_g = psA.tile([128, 4], f32, name="ps_g", tag="pga")
            for g in range(4):
                for q in range(16):
                    nc.tensor.matmul(
                        ps_g[:, g : g + 1],
                        wk_sb[q][:, 128 * g : 128 * (g + 1)],
                        z_sb[:, q : q + 1],
                        start=(q == 0),
                        stop=(q == 15),
                    )
            sig_i = spool.tile([128, 1], f32, name="sig_i")
            sig_f = spool.tile([128, 1], f32, name="sig_f")
            tanh_g = spool.tile([128, 1], f32, name="tanh_g")
            sig_o = spool.tile([128, 1], f32, name="sig_o")
            nc.scalar.activation(sig_i[:], ps_g[:, 0:1], AF.Sigmoid, bias=bk_sb[:, 0:1])
            nc.scalar.activation(sig_f[:], ps_g[:, 1:2], AF.Sigmoid, bias=bk_sb[:, 1:2])
            nc.scalar.activation(tanh_g[:], ps_g[:, 2:3], AF.Tanh, bias=bk_sb[:, 2:3])
            nc.scalar.activation(sig_o[:], ps_g[:, 3:4], AF.Sigmoid, bias=bk_sb[:, 3:4])
            t_fc = spool.tile([128, 1], f32, name="t_fc")
            t_ig = spool.tile([128, 1], f32, name="t_ig")
            c_new = spool.tile([128, 1], f32, name="c_new")
            tanh_c = spool.tile([128, 1], f32, name="tanh_c")
            ht_k = spool.tile([128, 1], f32, name="ht_k")
            ht_bf = spool.tile([128, 1], bf16, name="ht_bf")
            nc.vector.tensor_mul(t_fc[:], sig_f[:], c0_sb[:])
            nc.vector.tensor_mul(t_ig[:], sig_i[:], tanh_g[:])
            nc.vector.tensor_add(c_new[:], t_fc[:], t_ig[:])
            nc.scalar.activation(tanh_c[:], c_new[:], AF.Tanh)
            nc.vector.tensor_mul(ht_k[:], sig_o[:], tanh_c[:])
            nc.vector.tensor_copy(ht_bf[:], ht_k[:])
            nc.sync.dma_start(out_c[:, :], c_new[:])
            nc.sync.dma_start(out_h[:, :], ht_k[:])

            if stage < 2:
                return nc
            # ---------- A-partials + AllGather #1 (ht slices + A partials) ----------
            ps_a = psA.tile([128, 8], f32, name="ps_a", tag="pga")
            for m in range(8):
                nc.tensor.matmul(
                    ps_a[:, m : m + 1],
                    wl_sb[:, 128 * m : 128 * (m + 1)],
                    ht_bf[:],
                    start=True,
                    stop=True,
                )
            pay1 = spool.tile([128, 9], f32, name="pay1")
            nc.vector.tensor_copy(pay1[:, 0:1], ht_k[:])
            for m in range(8):
                nc.vector.tensor_add(
                    pay1[:, 1 + m : 2 + m], ps_a[:, m : m + 1], bs8_sb[:, m : m + 1]
                )
            ag1_in = dram.tile([128, 9], f32, name="ag1_in")
            ag1_out = dram.tile([128 * NCORES, 9], f32, addr_space="Shared" if collectives else "Local", name="ag1_out")
            nc.gpsimd.dma_start(ag1_in[:], pay1[:])
            if collectives:
                nc.gpsimd.collective_compute(
                    "AllGather",
                    mybir.AluOpType.bypass,
                    replica_groups=RG,
                    ins=[ag1_in[:].opt()],
                    outs=[ag1_out[:].opt()],
                )
            else:
                for c in range(NCORES):
                    nc.gpsimd.dma_start(ag1_out[128 * c : 128 * (c + 1), :], ag1_in[:])
            gath1 = spool.tile([128, NCORES, 9], f32, name="gath1")
            for c in range(NCORES):
                nc.gpsimd.dma_start(
                    gath1[:, c, :], ag1_out[128 * c : 128 * (c + 1), :]
                )

            if stage < 3:
                return nc
            # ---------- B matmul, scores (seq-sharded) ----------
            a_full = spool.tile([128, 8], f32, name="a_full")
            for m in range(8):
                nc.vector.reduce_sum(
                    a_full[:, m : m + 1], gath1[:, :, 1 + m], axis=mybir.AxisListType.X
                )
            ps_s = psC.tile([1, 512], f32, name="ps_s")
            for m in range(8):
                ps_b = psB.tile([128, 512], f32, name="ps_b", tag="psb")
                for q in range(8):
                    nc.tensor.matmul(
                        ps_b[:],
                        wr_sb[q][:, 128 * m : 128 * (m + 1)],
                        enc_sb[q][:],
                        start=(q == 0),
                        stop=(q == 7),
                    )
                sch = spool.tile([128, 512], bf16, name="sch", tag="sch", bufs=3)
                nc.scalar.activation(sch[:], ps_b[:], AF.Tanh, bias=a_full[:, m : m + 1])
                nc.tensor.matmul(
                    ps_s[:],
                    wva_sb[:, m : m + 1],
                    sch[:],
                    start=(m == 0),
                    stop=(m == 7),
                )

            if stage < 4:
                return nc
            # ---------- ht_tilda phase 1 (ht half; runs before AG2) ----------
            ht8_bf = spool.tile([128, 8], bf16, name="ht8_bf")
            nc.vector.tensor_copy(ht8_bf[:], gath1[:, :, 0])
            ps_ht1 = psA.tile([128, 8], f32, name="ps_ht1", tag="ht1")
            for m in range(8):
                for q in range(8, 16):
                    nc.tensor.matmul(
                        ps_ht1[:, m : m + 1],
                        wattn_sb[q][:, 128 * m : 128 * (m + 1)],
                        ht8_bf[:, q - 8 : q - 7],
                        start=(q == 8),
                        stop=(q == 15),
                    )

            # ---------- attention partial softmax (no max shift needed) ----------
            pay2 = spool.tile([128, 9], f32, name="pay2")
            nc.vector.memset(pay2[:, 8:9], 0.0)
            attn_u = spool.tile([128, 512], f32, name="attn_u")
            nc.vector.memset(attn_u[:], 0.0)
            nc.scalar.activation(attn_u[0:1, :], ps_s[:], AF.Exp)
            nc.vector.reduce_sum(pay2[0:1, 8:9], attn_u[0:1, :], axis=mybir.AxisListType.X)
            ps_rep = psB.tile([128, 512], f32, name="ps_rep", tag="psb")
            nc.tensor.matmul(ps_rep[:], ones128_sb[:], attn_u[:], start=True, stop=True)
            rep_bf = spool.tile([128, 512], bf16, name="rep_bf")
            nc.vector.tensor_copy(rep_bf[:], ps_rep[:])
            scr = spool.tile([128, 512], bf16, name="scr")
            for q in range(8):
                nc.vector.tensor_mul(scr[:], enc_sb[q][:], rep_bf[:])
                nc.vector.reduce_sum(pay2[:, q : q + 1], scr[:], axis=mybir.AxisListType.X)
            ag2_in = dram.tile([128, 9], f32, name="ag2_in")
            ag2_out = dram.tile([128 * NCORES, 9], f32, addr_space="Shared" if collectives else "Local", name="ag2_out")
            nc.gpsimd.dma_start(ag2_in[:], pay2[:])
            if collectives:
                nc.gpsimd.collective_compute(
                    "AllGather",
                    mybir.AluOpType.bypass,
                    replica_groups=RG,
                    ins=[ag2_in[:].opt()],
                    outs=[ag2_out[:].opt()],
                )
            else:
                for c in range(NCORES):
                    nc.gpsimd.dma_start(ag2_out[128 * c : 128 * (c + 1), :], ag2_in[:])
            gath2 = spool.tile([128, NCORES, 9], f32, name="gath2")
            for c in range(NCORES):
                nc.gpsimd.dma_start(
                    gath2[:, c, :], ag2_out[128 * c : 128 * (c + 1), :]
                )

            # ---------- global attention renorm, context ----------
            z_att = spool.tile([1, 1], f32, name="z_att")
            rz = spool.tile([128, 1], f32, name="rz")
            nc.vector.memset(rz[:], 0.0)
            nc.vector.reduce_sum(z_att[:], gath2[0:1, :, 8], axis=mybir.AxisListType.X)
            nc.vector.reciprocal(rz[0:1, 0:1], z_att[:])
            attn_f = spool.tile([1, 512], f32, name="attn_f")
            nc.vector.tensor_scalar_mul(attn_f[:], attn_u[0:1, :], rz[0:1, 0:1])
            nc.sync.dma_start(out_attn[:, :], attn_f[:])
            ct_raw = spool.tile([128, 8], f32, name="ct_raw")
            for q in range(8):
                nc.vector.reduce_sum(
                    ct_raw[:, q : q + 1], gath2[:, :, q], axis=mybir.AxisListType.X
                )
            ps_rz = psD.tile([128, 1], f32, name="ps_rz", tag="psd")
            nc.tensor.matmul(ps_rz[:], ones128_sb[:], rz[:], start=True, stop=True)
            rz_bc = spool.tile([128, 1], f32, name="rz_bc")
            nc.vector.tensor_copy(rz_bc[:], ps_rz[:])
            ct_bf = spool.tile([128, 8], bf16, name="ct_bf")
            nc.vector.tensor_scalar_mul(ct_bf[:], ct_raw[:], rz_bc[:])

            # ---------- ht_tilda phase 2 (ct half) + tanh ----------
            ps_ht2 = psA.tile([128, 8], f32, name="ps_ht2", tag="ht2")
            for m in range(8):
                for q in range(8):
                    nc.tensor.matmul(
                        ps_ht2[:, m : m + 1],
                        wattn_sb[q][:, 128 * m : 128 * (m + 1)],
                        ct_bf[:, q : q + 1],
                        start=(q == 0),
                        stop=(q == 7),
                    )
            htt_p1 = spool.tile([128, 8], f32, name="htt_p1")
            htt_pre = spool.tile([128, 8], f32, name="htt_pre")
            htt_bf = spool.tile([128, 8], bf16, name="htt_bf")
            nc.vector.tensor_add(htt_p1[:], ps_ht1[:], battn_sb[:])
            nc.vector.tensor_add(htt_pre[:], ps_ht2[:], htt_p1[:])
            nc.scalar.activation(htt_bf[:], htt_pre[:], AF.Tanh)

            if stage < 5:
                return nc
            # ---------- logits (vocab-sharded) + local sum-exp ----------
            ps_l = psB.tile([128, 32], f32, name="ps_l", tag="psb")
            for m in range(32):
                for h in range(8):
                    nc.tensor.matmul(
                        ps_l[:, m : m + 1],
                        wout_sb[h][:, 128 * m : 128 * (m + 1)],
                        htt_bf[:, h : h + 1],
                        start=(h == 0),
                        stop=(h == 7),
                    )
            logits_sb = spool.tile([128, 32], f32, name="logits_sb")
            nc.vector.tensor_add(logits_sb[:], ps_l[:], bout_sb[:])
            exp_col = spool.tile([128, 32], f32, name="exp_col")
            lsum_col = spool.tile([128, 1], f32, name="lsum_col")
            nc.scalar.activation(exp_col[:], logits_sb[:], AF.Exp)
            nc.vector.reduce_sum(lsum_col[:], exp_col[:], axis=mybir.AxisListType.X)
            ps_z = psD.tile([1, 1], f32, name="ps_z", tag="psd")
            nc.tensor.matmul(ps_z[:], onesc_sb[:], lsum_col[:], start=True, stop=True)
            z_loc = spool.tile([1, 64], f32, name="z_loc")
            nc.vector.memset(z_loc[:], 0.0)
            nc.vector.tensor_copy(z_loc[:, 0:1], ps_z[:])

            # ---------- AllGather #3: partial Z, global log-softmax ----------
            ag4_in = dram.tile([1, 64], f32, name="ag4_in")
            ag4_out = dram.tile([NCORES, 64], f32, addr_space="Shared" if collectives else "Local", name="ag4_out")
            nc.gpsimd.dma_start(ag4_in[:], z_loc[:])
            if collectives:
                nc.gpsimd.collective_compute(
                    "AllGather",
                    mybir.AluOpType.bypass,
                    replica_groups=RG,
                    ins=[ag4_in[:].opt()],
                    outs=[ag4_out[:].opt()],
                )
            else:
                for c in range(NCORES):
                    nc.gpsimd.dma_start(ag4_out[c : c + 1, :], ag4_in[:])
            g4 = spool.tile([1, NCORES], f32, name="g4")
            nc.gpsimd.dma_start(g4[:], ag4_out[:, 0:1].rearrange("c one -> one c"))
            zg = spool.tile([1, 1], f32, name="zg")
            nc.vector.reduce_sum(zg[:], g4[:], axis=mybir.AxisListType.X)
            logz = spool.tile([1, 1], f32, name="logz")
            nc.scalar.activation(logz[:], zg[:], AF.Ln)
            nlogz = spool.tile([128, 1], f32, name="nlogz")
            nc.vector.memset(nlogz[:], 0.0)
            nc.vector.tensor_scalar_mul(nlogz[0:1, 0:1], logz[:], -1.0)
            ps_nlz = psD.tile([128, 1], f32, name="ps_nlz", tag="psd")
            nc.tensor.matmul(ps_nlz[:], ones128_sb[:], nlogz[:], start=True, stop=True)
            nlz = spool.tile([128, 1], f32, name="nlz")
            nc.vector.tensor_copy(nlz[:], ps_nlz[:])
            out_col = spool.tile([128, 32], f32, name="out_col")
            nc.scalar.activation(out_col[:], logits_sb[:], AF.Identity, bias=nlz[:])

            # store column-layout tile to row-layout DRAM via strided AP
            nc.sync.dma_start(
                out_logits[:, :].rearrange("m p -> p m"), out_col[:]
            )

    return nc


def _shard_inputs(symbol, h0, c0, encoder_states, emb, W_ih, b_ih, W_hh, b_hh,
                  W_score, b_score, w_va, b_va, W_attn, b_attn, W_out, b_out):
    """Host-side sharding / layout prep. Returns in_maps (one dict per core)."""
    f32 = np.float32

    sym = int(np.asarray(symbol).reshape(-1)[0])
    x = np.asarray(emb)[sym].astype(f32).reshape(D)
    h0f = np.asarray(h0, dtype=f32).reshape(H)
    c0f = np.asarray(c0, dtype=f32).reshape(H)
    z = np.concatenate([x, h0f])                        # [2048]
    z_col = z.reshape(16, 128).T.astype(BF16)           # [128,16]

    W_cat = np.concatenate([np.asarray(W_ih, f32), np.asarray(W_hh, f32)], axis=1)  # [4H, 2048]
    b_cat = (np.asarray(b_ih, f32) + np.asarray(b_hh, f32))                         # [4H]

    W_score = np.asarray(W_score, f32)
    Wl = W_score[:, :H]                                  # [1024,1024]
    Wr_T = np.ascontiguousarray(W_score[:, H:].T).astype(BF16)   # [1024,1024]
    b_score = np.asarray(b_score, f32)
    w_va_f = np.asarray(w_va, f32).reshape(H)
    W_attn_T = np.ascontiguousarray(np.asarray(W_attn, f32).T).astype(BF16)  # [2048,1024]
    b_attn = np.asarray(b_attn, f32)
    W_out = np.asarray(W_out, f32)
    b_out = np.asarray(b_out, f32)
    enc = np.asarray(encoder_states, f32)

    ones_row = np.ones((1, 128), f32)
    negones_row = -np.ones((1, 128), f32)
    ones_col = np.ones((128, 1), f32)
    ident = np.eye(128, dtype=f32)
    bs8 = (b_score / NCORES).reshape(8, 128).T.copy()   # [128,8]
    battn_col = b_attn.reshape(8, 128).T.copy()
    wva_col = w_va_f.reshape(8, 128).T.astype(BF16)

    in_maps = []
    for k in range(NCORES):
        rows = np.concatenate([np.arange(j * H + 128 * k, j * H + 128 * (k + 1))
                               for j in range(4)])
        wk_T = np.ascontiguousarray(W_cat[rows].T).astype(BF16)          # [2048,512]
        bk_col = b_cat[rows].reshape(4, 128).T.copy()                    # [128,4]
        c0_col = c0f[128 * k : 128 * (k + 1)].reshape(128, 1).copy()
        wl_T = np.ascontiguousarray(Wl[:, 128 * k : 128 * (k + 1)].T).astype(BF16)  # [128,1024]
        encT = np.ascontiguousarray(enc[SS * k : SS * (k + 1)].T).astype(BF16)      # [1024,512]
        wo = W_out[VS * k : VS * (k + 1)]                                # [4000,1024]
        wo_T = np.zeros((1024, VSP), BF16)
        wo_T[:, :VS] = wo.T.astype(BF16)
        bo = np.full(VSP, -1e30, f32)
        bo[:VS] = b_out[VS * k : VS * (k + 1)]
        bout_col = bo.reshape(32, 128).T.copy()                          # [128,32]

        in_maps.append({
            "z_col": np.ascontiguousarray(z_col),
            "wk_T": wk_T,
            "bk_col": np.ascontiguousarray(bk_col),
            "c0_col": c0_col,
            "wl_T": wl_T,
            "bs8_col": np.ascontiguousarray(bs8),
            "wr_T": Wr_T,
            "encT": encT,
            "wva_col": np.ascontiguousarray(wva_col),
            "wattn_T": W_attn_T,
            "battn_col": np.ascontiguousarray(battn_col),
            "wout_T": wo_T,
            "bout_col": np.ascontiguousarray(bout_col),
            "ones_row": ones_row,
            "negones_row": negones_row,
            "ones_col": ones_col,
            "ones128": np.ones((128, 128), f32),
            "ident": ident,
        })
    return in_maps


def kernel(**inputs):
    from concourse import bass_utils

    if "nc" not in _CACHE:
        _CACHE["nc"] = _build_program()
    nc = _CACHE["nc"]

    in_maps = _shard_inputs(**inputs)
    res = bass_utils.run_bass_kernel_spmd(
        nc, in_maps, core_ids=list(range(NCORES)),
        trace=bool(_CACHE.get("trace", False)),
        tmpdir=_CACHE.get("tmpdir"),
    )
    _CACHE["last_result"] = res

    outs = res.results
    logits = np.concatenate(
        [outs[k]["out_logits"].reshape(VSP)[:VS] for k in range(NCORES)]
    ).reshape(1, V)
    next_h = np.concatenate(
        [outs[k]["out_h"].reshape(HS) for k in range(NCORES)]
    ).reshape(1, 1, H)
    next_c = np.concatenate(
        [outs[k]["out_c"].reshape(HS) for k in range(NCORES)]
    ).reshape(1, 1, H)
    attn = np.concatenate(
        [outs[k]["out_attn"].reshape(SS) for k in range(NCORES)]
    ).reshape(1, S)
    return (
        logits.astype(np.float32),
        next_h.astype(np.float32),
        next_c.astype(np.float32),
        attn.astype(np.float32),
    )


# revision 17
# speedup vs baseline: 1.1788x; 1.1788x over previous
"""Trainium2 Bass kernel for nn_Decoder (single-step attention decoder).

Sharding over 8 NeuronCores:
  - LSTM: gate-interleaved row shard (core k computes h/c slice [128k:128k+128])
  - score linear: A-term (ht part) input-sharded, B-term (encoder part)
    sequence-sharded; partial softmax with global renorm via AllGather
  - W_attn: replicated (bf16), ht_tilda computed fully on every core
  - W_out: vocab-sharded (4000 rows/core, padded to 4096); log-softmax via
    local sum-exp + tiny AllGather of partial Z
All weight matmuls run in bf16; accumulation/bias/softmax math in fp32.
Host-side work is limited to slicing / transposing / casting inputs and
reassembling outputs (the embedding row gather is part of sharding).
"""

import numpy as np
import ml_dtypes

V = 32000
D = 1024
H = 1024
E = 1024
S = 4096
NCORES = 8
HS = H // NCORES          # 128  hidden slice
SS = S // NCORES          # 512  sequence slice
VS = V // NCORES          # 4000 vocab slice
VSP = 4096                # padded vocab slice (32 x 128)

BF16 = ml_dtypes.bfloat16

_CACHE = {}


def _build_program(collectives=True, stage=5):
    nc = _build_body(collectives, stage)
    nc.compile()
    return nc


def _build_body(collectives=True, stage=5):
    import concourse.bacc as bacc
    import concourse.bass as bass
    import concourse.mybir as mybir
    import concourse.tile as tile

    f32 = mybir.dt.float32
    bf16 = mybir.dt.bfloat16
    AF = mybir.ActivationFunctionType

    nc = bacc.Bacc(
        "TRN2",
        target_bir_lowering=False,
        debug=False,
        num_devices=NCORES,
    )

    # ---------------- DRAM I/O ----------------
    din = {}

    def inp(name, shape, dt=bf16):
        din[name] = nc.dram_tensor(name, list(shape), dt, kind="ExternalInput").ap()
        return din[name]

    z_col = inp("z_col", [128, 16])                # [x; h0] column chunks
    wk_T = inp("wk_T", [2048, 512])                # LSTM weight slice (in-major)
    bk_col = inp("bk_col", [128, 4], f32)          # per-gate bias cols
    c0_col = inp("c0_col", [128, 1], f32)
    wl_T = inp("wl_T", [128, 1024])                # W_score[:, :H][:, hslice].T
    bs8_col = inp("bs8_col", [128, 8], f32)        # b_score/8 cols
    wr_T = inp("wr_T", [1024, 1024])               # W_score[:, H:].T (replicated)
    encT = inp("encT", [1024, 512])                # encoder slice transposed
    wva_col = inp("wva_col", [128, 8])             # w_va column chunks
    wattn_T = inp("wattn_T", [2048, 1024])         # W_attn.T (replicated)
    battn_col = inp("battn_col", [128, 8], f32)
    wout_T = inp("wout_T", [1024, VSP])            # W_out[vslice].T, padded
    bout_col = inp("bout_col", [128, 32], f32)     # b_out cols (pad = -1e30)
    ones_row = inp("ones_row", [1, 128], f32)
    negones_row = inp("negones_row", [1, 128], f32)
    ones_col = inp("ones_col", [128, 1], f32)
    ones128 = inp("ones128", [128, 128], f32)
    ident = inp("ident", [128, 128], f32)

    out_logits = nc.dram_tensor("out_logits", [32, 128], f32, kind="ExternalOutput").ap()
    out_h = nc.dram_tensor("out_h", [128, 1], f32, kind="ExternalOutput").ap()
    out_c = nc.dram_tensor("out_c", [128, 1], f32, kind="ExternalOutput").ap()
    out_attn = nc.dram_tensor("out_attn", [1, 512], f32, kind="ExternalOutput").ap()

    RG = [list(range(NCORES))]

    with tile.TileContext(nc) as tc:
        with (
            tc.tile_pool(name="wpool", bufs=1) as wpool,
            tc.tile_pool(name="spool", bufs=1) as spool,
            tc.tile_pool(name="dram", bufs=1, space="DRAM") as dram,
            tc.tile_pool(name="psA", bufs=1, space="PSUM") as psA,
            tc.tile_pool(name="psB", bufs=2, space="PSUM") as psB,
            tc.tile_pool(name="psC", bufs=1, space="PSUM") as psC,
            tc.tile_pool(name="psD", bufs=1, space="PSUM") as psD,
        ):
            # ---------- weight / constant loads (emitted in consumption order) ----------
            def load(pool, ap, shape, dt=bf16, name=None):
                t = pool.tile(shape, dt, name=name, uniquify=True)
                nc.sync.dma_start(t[:], ap)
                return t

            z_sb = load(spool, z_col[:, :], [128, 16], name="z_sb")
            bk_sb = load(spool, bk_col[:, :], [128, 4], f32, name="bk_sb")
            c0_sb = load(spool, c0_col[:, :], [128, 1], f32, name="c0_sb")
            wk_sb = [
                load(wpool, wk_T[128 * q : 128 * (q + 1), :], [128, 512], name=f"wk{q}")
                for q in range(16)
            ]
            wl_sb = load(wpool, wl_T[:, :], [128, 1024], name="wl_sb")
            bs8_sb = load(spool, bs8_col[:, :], [128, 8], f32, name="bs8_sb")
            ones_sb = load(spool, ones_row[:, :], [1, 128], f32, name="ones_sb")
            negones_sb = load(spool, negones_row[:, :], [1, 128], f32, name="negones_sb")
            onesc_sb = load(spool, ones_col[:, :], [128, 1], f32, name="onesc_sb")
            ones128_sb = load(spool, ones128[:, :], [128, 128], f32, name="ones128_sb")
            ident_sb = load(spool, ident[:, :], [128, 128], f32, name="ident_sb")
            wva_sb = load(spool, wva_col[:, :], [128, 8], name="wva_sb")
            wr_sb = [
                load(wpool, wr_T[128 * q : 128 * (q + 1), :], [128, 1024], name=f"wr{q}")
                for q in range(8)
            ]
            enc_sb = [
                load(wpool, encT[128 * q : 128 * (q + 1), :], [128, 512], name=f"enc{q}")
                for q in range(8)
            ]
            wattn_sb = [
                load(wpool, wattn_T[128 * q : 128 * (q + 1), :], [128, 1024], name=f"wat{q}")
                for q in range(16)
            ]
            battn_sb = load(spool, battn_col[:, :], [128, 8], f32, name="battn_sb")
            bout_sb = load(spool, bout_col[:, :], [128, 32], f32, name="bout_sb")
            wout_sb = [
                load(wpool, wout_T[128 * q : 128 * (q + 1), :], [128, VSP], name=f"wo{q}")
                for q in range(8)
            ]

            # ---------- LSTM step (local 128-slice of h and c) ----------
            ps_g = psA.tile([128, 4], f32, name="ps_g", tag="pga")
            for g in range(4):
                for q in range(16):
                    nc.tensor.matmul(
                        ps_g[:, g : g + 1],
                        wk_sb[q][:, 128 * g : 128 * (g + 1)],
                        z_sb[:, q : q + 1],
                        start=(q == 0),
                        stop=(q == 15),
                    )
            sig_i = spool.tile([128, 1], f32, name="sig_i")
            sig_f = spool.tile([128, 1], f32, name="sig_f")
            tanh_g = spool.tile([128, 1], f32, name="tanh_g")
            sig_o = spool.tile([128, 1], f32, name="sig_o")
            nc.scalar.activation(sig_i[:], ps_g[:, 0:1], AF.Sigmoid, bias=bk_sb[:, 0:1])
            nc.scalar.activation(sig_f[:], ps_g[:, 1:2], AF.Sigmoid, bias=bk_sb[:, 1:2])
            nc.scalar.activation(tanh_g[:], ps_g[:, 2:3], AF.Tanh, bias=bk_sb[:, 2:3])
            nc.scalar.activation(sig_o[:], ps_g[:, 3:4], AF.Sigmoid, bias=bk_sb[:, 3:4])
            t_fc = spool.tile([128, 1], f32, name="t_fc")
            t_ig = spool.tile([128, 1], f32, name="t_ig")
            c_new = spool.tile([128, 1], f32, name="c_new")
            tanh_c = spool.tile([128, 1], f32, name="tanh_c")
            ht_k = spool.tile([128, 1], f32, name="ht_k")
            ht_bf = spool.tile([128, 1], bf16, name="ht_bf")
            nc.vector.tensor_mul(t_fc[:], sig_f[:], c0_sb[:])
            nc.vector.tensor_mul(t_ig[:], sig_i[:], tanh_g[:])
            nc.vector.tensor_add(c_new[:], t_fc[:], t_ig[:])
            nc.scalar.activation(tanh_c[:], c_new[:], AF.Tanh)
            nc.vector.tensor_mul(ht_k[:], sig_o[:], tanh_c[:])
            nc.vector.tensor_copy(ht_bf[:], ht_k[:])
            nc.sync.dma_start(out_c[:, :], c_new[:])
            nc.sync.dma_start(out_h[:, :], ht_k[:])

            if stage < 2:
                return nc
            # ---------- A-partials + AllGather #1 (ht slices + A partials) ----------
            ps_a = psA.tile([128, 8], f32, name="ps_a", tag="pga")
            for m in range(8):
                nc.tensor.matmul(
                    ps_a[:, m : m + 1],
                    wl_sb[:, 128 * m : 128 * (m + 1)],
                    ht_bf[:],
                    start=True,
                    stop=True,
                )
            pay1 = spool.tile([128, 9], f32, name="pay1")
            nc.vector.tensor_copy(pay1[:, 0:1], ht_k[:])
            for m in range(8):
                nc.vector.tensor_add(
                    pay1[:, 1 + m : 2 + m], ps_a[:, m : m + 1], bs8_sb[:, m : m + 1]
                )
            ag1_in = dram.tile([128, 9], f32, name="ag1_in")
            ag1_out = dram.tile([128 * NCORES, 9], f32, addr_space="Shared" if collectives else "Local", name="ag1_out")
            nc.gpsimd.dma_start(ag1_in[:], pay1[:])
            if collectives:
                nc.gpsimd.collective_compute(
                    "AllGather",
                    mybir.AluOpType.bypass,
                    replica_groups=RG,
                    ins=[ag1_in[:].opt()],
                    outs=[ag1_out[:].opt()],
                )
            else:
                for c in range(NCORES):
                    nc.gpsimd.dma_start(ag1_out[128 * c : 128 * (c + 1), :], ag1_in[:])
            gath1 = spool.tile([128, NCORES, 9], f32, name="gath1")
            for c in range(NCORES):
                nc.gpsimd.dma_start(
                    gath1[:, c, :], ag1_out[128 * c : 128 * (c + 1), :]
                )

            if stage < 3:
                return nc
            # ---------- B matmul, scores (seq-sharded) ----------
            a_full = spool.tile([128, 8], f32, name="a_full")
            for m in range(8):
                nc.vector.reduce_sum(
                    a_full[:, m : m + 1], gath1[:, :, 1 + m], axis=mybir.AxisListType.X
                )
            ps_s = psC.tile([1, 512], f32, name="ps_s")
            b_sb = []
            for m in range(8):
                ps_b = psB.tile([128, 512], f32, name="ps_b", tag="psb")
                for q in range(8):
                    nc.tensor.matmul(
                        ps_b[:],
                        wr_sb[q][:, 128 * m : 128 * (m + 1)],
                        enc_sb[q][:],
                        start=(q == 0),
                        stop=(q == 7),
                    )
                bt = spool.tile([128, 512], f32, name=f"b_sb{m}")
                nc.vector.tensor_copy(bt[:], ps_b[:])
                b_sb.append(bt)
            for m in range(8):
                sch = spool.tile([128, 512], bf16, name="sch", tag="sch", bufs=3)
                nc.scalar.activation(sch[:], b_sb[m][:], AF.Tanh, bias=a_full[:, m : m + 1])
                nc.tensor.matmul(
                    ps_s[:],
                    wva_sb[:, m : m + 1],
                    sch[:],
                    start=(m == 0),
                    stop=(m == 7),
                )

            if stage < 4:
                return nc
            # ---------- ht_tilda phase 1 (ht half; runs before AG2) ----------
            ht8_bf = spool.tile([128, 8], bf16, name="ht8_bf")
            nc.vector.tensor_copy(ht8_bf[:], gath1[:, :, 0])
            ps_ht1 = psA.tile([128, 8], f32, name="ps_ht1", tag="ht1")
            for m in range(8):
                for q in range(8, 16):
                    nc.tensor.matmul(
                        ps_ht1[:, m : m + 1],
                        wattn_sb[q][:, 128 * m : 128 * (m + 1)],
                        ht8_bf[:, q - 8 : q - 7],
                        start=(q == 8),
                        stop=(q == 15),
                    )

            # ---------- attention partial softmax (no max shift needed) ----------
            pay2 = spool.tile([128, 9], f32, name="pay2")
            nc.vector.memset(pay2[:, 8:9], 0.0)
            attn_u = spool.tile([128, 512], f32, name="attn_u")
            nc.vector.memset(attn_u[:], 0.0)
            nc.scalar.activation(attn_u[0:1, :], ps_s[:], AF.Exp)
            nc.vector.reduce_sum(pay2[0:1, 8:9], attn_u[0:1, :], axis=mybir.AxisListType.X)
            ps_rep = psB.tile([128, 512], f32, name="ps_rep", tag="psb")
            nc.tensor.matmul(ps_rep[:], ones128_sb[:], attn_u[:], start=True, stop=True)
            rep_bf = spool.tile([128, 512], bf16, name="rep_bf")
            nc.vector.tensor_copy(rep_bf[:], ps_rep[:])
            for h in range(40):
                ps_heat = psB.tile([128, 512], f32, name="ps_heat", tag="psb")
                nc.tensor.matmul(
                    ps_heat[:], wr_sb[0][:, 0:128], enc_sb[0][:], start=True, stop=True
                )
            scr = spool.tile([128, 512], bf16, name="scr")
            for q in range(8):
                nc.vector.tensor_mul(scr[:], enc_sb[q][:], rep_bf[:])
                nc.vector.reduce_sum(pay2[:, q : q + 1], scr[:], axis=mybir.AxisListType.X)
            ag2_in = dram.tile([128, 9], f32, name="ag2_in")
            ag2_out = dram.tile([128 * NCORES, 9], f32, addr_space="Shared" if collectives else "Local", name="ag2_out")
            nc.gpsimd.dma_start(ag2_in[:], pay2[:])
            if collectives:
                nc.gpsimd.collective_compute(
                    "AllGather",
                    mybir.AluOpType.bypass,
                    replica_groups=RG,
                    ins=[ag2_in[:].opt()],
                    outs=[ag2_out[:].opt()],
                )
            else:
                for c in range(NCORES):
                    nc.gpsimd.dma_start(ag2_out[128 * c : 128 * (c + 1), :], ag2_in[:])
            gath2 = spool.tile([128, NCORES, 9], f32, name="gath2")
            for c in range(NCORES):
                nc.gpsimd.dma_start(
                    gath2[:, c, :], ag2_out[128 * c : 128 * (c + 1), :]
                )

            # ---------- global attention renorm, context ----------
            z_att = spool.tile([1, 1], f32, name="z_att")
            rz = spool.tile([128, 1], f32, name="rz")
            nc.vector.memset(rz[:], 0.0)
            nc.vector.reduce_sum(z_att[:], gath2[0:1, :, 8], axis=mybir.AxisListType.X)
            nc.vector.reciprocal(rz[0:1, 0:1], z_att[:])
            attn_f = spool.tile([1, 512], f32, name="attn_f")
            nc.vector.tensor_scalar_mul(attn_f[:], attn_u[0:1, :], rz[0:1, 0:1])
            nc.sync.dma_start(out_attn[:, :], attn_f[:])
            ct_raw = spool.tile([128, 8], f32, name="ct_raw")
            for q in range(8):
                nc.vector.reduce_sum(
                    ct_raw[:, q : q + 1], gath2[:, :, q], axis=mybir.AxisListType.X
                )
            ps_rz = psD.tile([128, 1], f32, name="ps_rz", tag="psd")
            nc.tensor.matmul(ps_rz[:], ones128_sb[:], rz[:], start=True, stop=True)
            rz_bc = spool.tile([128, 1], f32, name="rz_bc")
            nc.vector.tensor_copy(rz_bc[:], ps_rz[:])
            ct_bf = spool.tile([128, 8], bf16, name="ct_bf")
            nc.vector.tensor_scalar_mul(ct_bf[:], ct_raw[:], rz_bc[:])

            # ---------- ht_tilda phase 2 (ct half) + tanh ----------
            ps_ht2 = psA.tile([128, 8], f32, name="ps_ht2", tag="ht2")
            for m in range(8):
                for q in range(8):
                    nc.tensor.matmul(
                        ps_ht2[:, m : m + 1],
                        wattn_sb[q][:, 128 * m : 128 * (m + 1)],
                        ct_bf[:, q : q + 1],
                        start=(q == 0),
                        stop=(q == 7),
                    )
            htt_p1 = spool.tile([128, 8], f32, name="htt_p1")
            htt_pre = spool.tile([128, 8], f32, name="htt_pre")
            htt_bf = spool.tile([128, 8], bf16, name="htt_bf")
            nc.vector.tensor_add(htt_p1[:], ps_ht1[:], battn_sb[:])
            nc.vector.tensor_add(htt_pre[:], ps_ht2[:], htt_p1[:])
            nc.scalar.activation(htt_bf[:], htt_pre[:], AF.Tanh)

            if stage < 5:
                return nc
            # ---------- logits (vocab-sharded) + local sum-exp ----------
            ps_l = psB.tile([128, 32], f32, name="ps_l", tag="psb")
            for m in range(32):
                for h in range(8):
                    nc.tensor.matmul(
                        ps_l[:, m : m + 1],
                        wout_sb[h][:, 128 * m : 128 * (m + 1)],
                        htt_bf[:, h : h + 1],
                        start=(h == 0),
                        stop=(h == 7),
                    )
            logits_sb = spool.tile([128, 32], f32, name="logits_sb")
            nc.vector.tensor_add(logits_sb[:], ps_l[:], bout_sb[:])
            exp_col = spool.tile([128, 32], f32, name="exp_col")
            lsum_col = spool.tile([128, 1], f32, name="lsum_col")
            nc.scalar.activation(exp_col[:], logits_sb[:], AF.Exp)
            nc.vector.reduce_sum(lsum_col[:], exp_col[:], axis=mybir.AxisListType.X)
            ps_z = psD.tile([1, 1], f32, name="ps_z", tag="psd")
            nc.tensor.matmul(ps_z[:], onesc_sb[:], lsum_col[:], start=True, stop=True)
            z_loc = spool.tile([1, 64], f32, name="z_loc")
            nc.vector.memset(z_loc[:], 0.0)
            nc.vector.tensor_copy(z_loc[:, 0:1], ps_z[:])

            # ---------- AllGather #3: partial Z, global log-softmax ----------
            ag4_in = dram.tile([1, 64], f32, name="ag4_in")
            ag4_out = dram.tile([NCORES, 64], f32, addr_space="Shared" if collectives else "Local", name="ag4_out")
            nc.gpsimd.dma_start(ag4_in[:], z_loc[:])
            if collectives:
                nc.gpsimd.collective_compute(
                    "AllGather",
                    mybir.AluOpType.bypass,
                    replica_groups=RG,
                    ins=[ag4_in[:].opt()],
                    outs=[ag4_out[:].opt()],
                )
            else:
                for c in range(NCORES):
                    nc.gpsimd.dma_start(ag4_out[c : c + 1, :], ag4_in[:])
            g4 = spool.tile([1, NCORES], f32, name="g4")
            nc.gpsimd.dma_start(g4[:], ag4_out[:, 0:1].rearrange("c one -> one c"))
            zg = spool.tile([1, 1], f32, name="zg")
            nc.vector.reduce_sum(zg[:], g4[:], axis=mybir.AxisListType.X)
            logz = spool.tile([1, 1], f32, name="logz")
            nc.scalar.activation(logz[:], zg[:], AF.Ln)
            nlogz = spool.tile([128, 1], f32, name="nlogz")
            nc.vector.memset(nlogz[:], 0.0)
            nc.vector.tensor_scalar_mul(nlogz[0:1, 0:1], logz[:], -1.0)
            ps_nlz = psD.tile([128, 1], f32, name="ps_nlz", tag="psd")
            nc.tensor.matmul(ps_nlz[:], ones128_sb[:], nlogz[:], start=True, stop=True)
            nlz = spool.tile([128, 1], f32, name="nlz")
            nc.vector.tensor_copy(nlz[:], ps_nlz[:])
            out_col = spool.tile([128, 32], f32, name="out_col")
            nc.scalar.activation(out_col[:], logits_sb[:], AF.Identity, bias=nlz[:])

            ps_t = psD.tile([32, 128], f32, name="ps_t", tag="psd")
            nc.tensor.transpose(ps_t[:], out_col[:], ident_sb[:])
            outrow = spool.tile([32, 128], f32, name="outrow")
            nc.vector.tensor_copy(outrow[:], ps_t[:])
            nc.sync.dma_start(out_logits[:, :], outrow[:])

    return nc


def _shard_inputs(symbol, h0, c0, encoder_states, emb, W_ih, b_ih, W_hh, b_hh,
                  W_score, b_score, w_va, b_va, W_attn, b_attn, W_out, b_out):
    """Host-side sharding / layout prep. Returns in_maps (one dict per core)."""
    f32 = np.float32

    sym = int(np.asarray(symbol).reshape(-1)[0])
    x = np.asarray(emb)[sym].astype(f32).reshape(D)
    h0f = np.asarray(h0, dtype=f32).reshape(H)
    c0f = np.asarray(c0, dtype=f32).reshape(H)
    z = np.concatenate([x, h0f])                        # [2048]
    z_col = z.reshape(16, 128).T.astype(BF16)           # [128,16]

    W_cat = np.concatenate([np.asarray(W_ih, f32), np.asarray(W_hh, f32)], axis=1)  # [4H, 2048]
    b_cat = (np.asarray(b_ih, f32) + np.asarray(b_hh, f32))                         # [4H]

    W_score = np.asarray(W_score, f32)
    Wl = W_score[:, :H]                                  # [1024,1024]
    Wr_T = np.ascontiguousarray(W_score[:, H:].T).astype(BF16)   # [1024,1024]
    b_score = np.asarray(b_score, f32)
    w_va_f = np.asarray(w_va, f32).reshape(H)
    W_attn_T = np.ascontiguousarray(np.asarray(W_attn, f32).T).astype(BF16)  # [2048,1024]
    b_attn = np.asarray(b_attn, f32)
    W_out = np.asarray(W_out, f32)
    b_out = np.asarray(b_out, f32)
    enc = np.asarray(encoder_states, f32)

    ones_row = np.ones((1, 128), f32)
    negones_row = -np.ones((1, 128), f32)
    ones_col = np.ones((128, 1), f32)
    ident = np.eye(128, dtype=f32)
    bs8 = (b_score / NCORES).reshape(8, 128).T.copy()   # [128,8]
    battn_col = b_attn.reshape(8, 128).T.copy()
    wva_col = w_va_f.reshape(8, 128).T.astype(BF16)

    in_maps = []
    for k in range(NCORES):
        rows = np.concatenate([np.arange(j * H + 128 * k, j * H + 128 * (k + 1))
                               for j in range(4)])
        wk_T = np.ascontiguousarray(W_cat[rows].T).astype(BF16)          # [2048,512]
        bk_col = b_cat[rows].reshape(4, 128).T.copy()                    # [128,4]
        c0_col = c0f[128 * k : 128 * (k + 1)].reshape(128, 1).copy()
        wl_T = np.ascontiguousarray(Wl[:, 128 * k : 128 * (k + 1)].T).astype(BF16)  # [128,1024]
        encT = np.ascontiguousarray(enc[SS * k : SS * (k + 1)].T).astype(BF16)      # [1024,512]
        wo = W_out[VS * k : VS * (k + 1)]                                # [4000,1024]
        wo_T = np.zeros((1024, VSP), BF16)
        wo_T[:, :VS] = wo.T.astype(BF16)
        bo = np.full(VSP, -1e30, f32)
        bo[:VS] = b_out[VS * k : VS * (k + 1)]
        bout_col = bo.reshape(32, 128).T.copy()                          # [128,32]

        in_maps.append({
            "z_col": np.ascontiguousarray(z_col),
            "wk_T": wk_T,
            "bk_col": np.ascontiguousarray(bk_col),
            "c0_col": c0_col,
            "wl_T": wl_T,
            "bs8_col": np.ascontiguousarray(bs8),
            "wr_T": Wr_T,
            "encT": encT,
            "wva_col": np.ascontiguousarray(wva_col),
            "wattn_T": W_attn_T,
            "battn_col": np.ascontiguousarray(battn_col),
            "wout_T": wo_T,
            "bout_col": np.ascontiguousarray(bout_col),
            "ones_row": ones_row,
            "negones_row": negones_row,
            "ones_col": ones_col,
            "ones128": np.ones((128, 128), f32),
            "ident": ident,
        })
    return in_maps


def kernel(**inputs):
    from concourse import bass_utils

    if "nc" not in _CACHE:
        _CACHE["nc"] = _build_program()
    nc = _CACHE["nc"]

    in_maps = _shard_inputs(**inputs)
    res = bass_utils.run_bass_kernel_spmd(
        nc, in_maps, core_ids=list(range(NCORES)),
        trace=bool(_CACHE.get("trace", False)),
        tmpdir=_CACHE.get("tmpdir"),
    )
    _CACHE["last_result"] = res

    outs = res.results
    logits = np.concatenate(
        [outs[k]["out_logits"].reshape(VSP)[:VS] for k in range(NCORES)]
    ).reshape(1, V)
    next_h = np.concatenate(
        [outs[k]["out_h"].reshape(HS) for k in range(NCORES)]
    ).reshape(1, 1, H)
    next_c = np.concatenate(
        [outs[k]["out_c"].reshape(HS) for k in range(NCORES)]
    ).reshape(1, 1, H)
    attn = np.concatenate(
        [outs[k]["out_attn"].reshape(SS) for k in range(NCORES)]
    ).reshape(1, S)
    return (
        logits.astype(np.float32),
        next_h.astype(np.float32),
        next_c.astype(np.float32),
        attn.astype(np.float32),
    )
